# revision 1
# baseline (speedup 1.0000x reference)
"""MoNet (GMM graph conv) 3-layer kernel for one TRN2 chip (8 NeuronCores).

Strategy (graph/data parallel, dst-sharded):
  - Nodes are split into 8 contiguous shards of 2500; core c owns all edges
    whose dst lands in its shard (host-side index prep only).
  - Per layer, each core:
      * computes Gaussian mixture weights w[e,k] on device (DVE+ACT),
      * dma_gather's h[src[e]] rows (bf16, 256B rows) from a replicated
        full-h DRAM table,
      * aggregates g_k[n] = sum_e 1[dst=n] * w[e,k] * h[src[e]] with a
        one-hot "mask matmul" on the tensor engine (PSUM accumulation over
        128-edge tiles, node-tile = 128 dst nodes),
      * applies the dense transform agg = sum_k g_k @ W_k + bias
        (PE transposes + 4 accumulated matmuls),
      * AllGather's the new h shard to every core (bf16).
  - Compute dtype bf16 (fp32 PSUM accumulation); w computed in fp32.
"""

import sys

sys.path.insert(0, "/opt/trn_rl_repo")

import numpy as np
import ml_dtypes

from concourse import bacc, mybir
from concourse import tile
from concourse.bass_utils import run_bass_kernel_spmd
from concourse.library_config import mlp

import os
N_LAYERS = int(os.environ.get("KERN_LAYERS", "3"))
USE_CC = os.environ.get("KERN_CC", "1") == "1"
N_BINS = int(os.environ.get("KERN_BINS", "99"))
SKIP_W = os.environ.get("KERN_SKIPW", "0") == "1"
GSPLIT = os.environ.get("KERN_GSPLIT", "0") == "1"

N_NODES = 20000
N_EDGES = 320000
IN_FEATS = 64
D = 128            # padded feature width, = hidden width for all layers
K = 4
N_CORES = 8
SHARD = N_NODES // N_CORES          # 2500
NT = (SHARD + 127) // 128           # 20 node tiles per core (last has 68 rows)
BF = mybir.dt.bfloat16
F32 = mybir.dt.float32
I16 = mybir.dt.int16
bf16 = ml_dtypes.bfloat16


def _plan_edges(src, dst):
    """Partition + sort + pad edges. Returns per-core index arrays and the
    shared per-node-tile tile counts T_bins (identical across cores so the
    single SPMD program fits every core)."""
    core_of = dst // SHARD
    plans = []
    counts = np.zeros((N_CORES, NT), dtype=np.int64)
    per_core = []
    for c in range(N_CORES):
        sel = np.nonzero(core_of == c)[0]
        dl = dst[sel] - c * SHARD
        nt = dl // 128
        order = np.argsort(nt, kind="stable")
        sel, dl, nt = sel[order], dl[order], nt[order]
        per_core.append((sel, dl, nt))
        counts[c] = np.bincount(nt, minlength=NT)
    T_bins = np.maximum(1, (counts.max(axis=0) + 127) // 128).astype(np.int64)
    T_tot = int(T_bins.sum())
    for c in range(N_CORES):
        sel, dl, nt = per_core[c]
        srcP = np.zeros(T_tot * 128, dtype=np.int64)
        dstlocP = np.full(T_tot * 128, -1.0, dtype=np.float32)
        origP = np.full(T_tot * 128, -1, dtype=np.int64)
        tbase = 0
        pos = 0
        for b in range(NT):
            n = int(counts[c, b])
            lo = tbase * 128
            srcP[lo : lo + n] = src[sel[pos : pos + n]]
            dstlocP[lo : lo + n] = (dl[pos : pos + n] - b * 128).astype(np.float32)
            origP[lo : lo + n] = sel[pos : pos + n]
            pos += n
            tbase += int(T_bins[b])
        plans.append((srcP, dstlocP, origP))
    return T_bins, T_tot, plans


def _wrap_idx(idx_flat):
    """[n] int -> [128, n//16] int16 gather-index layout (16-partition wrap,
    replicated across the 8 Q7 cores)."""
    n = idx_flat.shape[0]
    w = idx_flat.reshape(n // 16, 16).T.astype(np.int16)
    return np.tile(w, (8, 1)).copy()


def _rep(v, cols=None):
    """Replicate a scalar/vector across 128 partitions as float32."""
    v = np.asarray(v, dtype=np.float32).reshape(-1)
    return np.tile(v, (128, 1)).copy()


def build_program(T_bins, T_tot):
    nc = bacc.Bacc("TRN2", target_bir_lowering=False, debug=False,
                   num_devices=N_CORES)

    featP_d = nc.dram_tensor("featP", [128, T_tot, D], BF, kind="ExternalInput")
    idx_d = nc.dram_tensor("idx", [128, T_tot * 8], I16, kind="ExternalInput")
    dstloc_d = nc.dram_tensor("dstloc", [128, T_tot], BF, kind="ExternalInput")
    pseudo_d = nc.dram_tensor("pseudo", [128, T_tot, 2], F32, kind="ExternalInput")
    iota_d = nc.dram_tensor("iota", [128, 128], BF, kind="ExternalInput")
    ident_d = nc.dram_tensor("ident", [128, 128], BF, kind="ExternalInput")
    fcw_d, pw_d, pb_d, mu_d, isg_d, bias_d = [], [], [], [], [], []
    for l in range(3):
        fcw_d.append(nc.dram_tensor(f"fcw{l}", [128, K, D], BF, kind="ExternalInput"))
        pw_d.append(nc.dram_tensor(f"pw{l}", [128, 4], F32, kind="ExternalInput"))
        pb_d.append(nc.dram_tensor(f"pb{l}", [128, 2], F32, kind="ExternalInput"))
        mu_d.append(nc.dram_tensor(f"mu{l}", [128, 2 * K], F32, kind="ExternalInput"))
        isg_d.append(nc.dram_tensor(f"isg{l}", [128, 2 * K], F32, kind="ExternalInput"))
        bias_d.append(nc.dram_tensor(f"bias{l}", [128, D], F32, kind="ExternalInput"))
    out_d = nc.dram_tensor("out", [SHARD, D], F32, kind="ExternalOutput")

    AF = mybir.ActivationFunctionType
    OP = mybir.AluOpType

    with tile.TileContext(nc) as tc:
        with (
            tc.tile_pool(name="const", bufs=1) as cpool,
            tc.tile_pool(name="wrk", bufs=2) as wpool,
            tc.tile_pool(name="hbin", bufs=3) as hpool,
            tc.tile_pool(name="maskp", bufs=3) as mpool,
            tc.tile_pool(name="scp", bufs=4) as spool,
            tc.tile_pool(name="outp", bufs=3) as opool,
            tc.tile_pool(name="gps", bufs=2, space="PSUM") as gpsum,
            tc.tile_pool(name="tps", bufs=2, space="PSUM") as tpsum,
            tc.tile_pool(name="aps", bufs=2, space="PSUM") as apsum,
            tc.tile_pool(name="dram", bufs=1, space="DRAM") as dram,
        ):
            nc.gpsimd.load_library(mlp)

            idx_sb = cpool.tile([128, T_tot * 8], I16)
            dstloc = cpool.tile([128, T_tot], BF)
            pseudo = cpool.tile([128, T_tot, 2], F32)
            iota = cpool.tile([128, 128], BF)
            ident = cpool.tile([128, 128], BF)
            nc.sync.dma_start(idx_sb[:], idx_d[:])
            nc.sync.dma_start(dstloc[:], dstloc_d[:])
            nc.sync.dma_start(pseudo[:], pseudo_d[:])
            nc.sync.dma_start(iota[:], iota_d[:])
            nc.sync.dma_start(ident[:], ident_d[:])
            fcw, pwt, pbt, mut, isgt, biast = [], [], [], [], [], []
            for l in range(3):
                fcw.append(cpool.tile([128, K, D], BF, tag=f"fcw{l}", name=f"fcw{l}"))
                pwt.append(cpool.tile([128, 4], F32, tag=f"pw{l}", name=f"pwt{l}"))
                pbt.append(cpool.tile([128, 2], F32, tag=f"pb{l}", name=f"pbt{l}"))
                mut.append(cpool.tile([128, 2 * K], F32, tag=f"mu{l}", name=f"mut{l}"))
                isgt.append(cpool.tile([128, 2 * K], F32, tag=f"isg{l}", name=f"isgt{l}"))
                biast.append(cpool.tile([128, D], F32, tag=f"bias{l}", name=f"biast{l}"))
                nc.sync.dma_start(fcw[l][:], fcw_d[l][:])
                nc.sync.dma_start(pwt[l][:], pw_d[l][:])
                nc.sync.dma_start(pbt[l][:], pb_d[l][:])
                nc.sync.dma_start(mut[l][:], mu_d[l][:])
                nc.sync.dma_start(isgt[l][:], isg_d[l][:])
                nc.sync.dma_start(biast[l][:], bias_d[l][:])

            # DRAM bounce buffers for the inter-layer AllGather
            shard_t = [dram.tile([SHARD, D], BF, tag=f"shard{l}", name=f"shard{l}") for l in range(2)]
            hag_t = [dram.tile([N_NODES, D], BF, tag=f"hag{l}", name=f"hag{l}") for l in range(2)]

            for l in range(N_LAYERS):
                hsrc = None if (l == 0 or not USE_CC) else hag_t[l - 1][:]

                # ---- Phase W: mixture weights w[e,k] for every edge slot ----
                w_all = wpool.tile([128, K, T_tot], F32, tag="w_all")
                u = wpool.tile([128, 2, T_tot], F32, tag="u")
                tmp0 = wpool.tile([128, T_tot], F32, tag="tmp0")
                tmp1 = wpool.tile([128, T_tot], F32, tag="tmp1")
                if SKIP_W:
                    nc.vector.memset(w_all[:], 0.5)
                for d in range(2 if not SKIP_W else 0):
                    # u_d = tanh(p0*pw[0,d] + p1*pw[1,d] + pb[d])
                    nc.vector.tensor_scalar(tmp0[:], pseudo[:, :, 0],
                                            pwt[l][:, d : d + 1], None, OP.mult)
                    nc.vector.tensor_scalar(tmp1[:], pseudo[:, :, 1],
                                            pwt[l][:, 2 + d : 3 + d], None, OP.mult)
                    nc.vector.tensor_tensor(tmp0[:], tmp0[:], tmp1[:], OP.add)
                    nc.scalar.activation(u[:, d, :], tmp0[:], AF.Tanh,
                                         bias=pbt[l][:, d : d + 1])
                for k in range(K if not SKIP_W else 0):
                    nc.vector.tensor_scalar(tmp0[:], u[:, 0, :],
                                            mut[l][:, 2 * k : 2 * k + 1],
                                            isgt[l][:, 2 * k : 2 * k + 1],
                                            OP.subtract, OP.mult)
                    nc.vector.tensor_scalar(tmp1[:], u[:, 1, :],
                                            mut[l][:, 2 * k + 1 : 2 * k + 2],
                                            isgt[l][:, 2 * k + 1 : 2 * k + 2],
                                            OP.subtract, OP.mult)
                    nc.vector.tensor_tensor(tmp0[:], tmp0[:], tmp0[:], OP.mult)
                    nc.vector.tensor_tensor(tmp1[:], tmp1[:], tmp1[:], OP.mult)
                    nc.vector.tensor_tensor(tmp0[:], tmp0[:], tmp1[:], OP.add)
                    nc.scalar.activation(w_all[:, k, :], tmp0[:], AF.Exp, scale=-0.5)

                # ---- Phase E: per node-tile gather + mask-matmul + transform ----
                # gather chunks of CH tiles (dma_gather caps at 1024 idxs);
                # layer 0 reads host-pre-gathered rows contiguously instead.
                CH = 8
                chunks = {}

                def get_chunk(t):
                    c = t // CH
                    if c not in chunks:
                        n = min(CH, T_tot - c * CH)
                        Hc = hpool.tile([128, CH, D], BF, tag="hbin",
                                        name=f"hb_{l}_{c}")
                        if hsrc is None:
                            nc.sync.dma_start(Hc[:, :n, :],
                                              featP_d[:, c * CH : c * CH + n, :])
                        else:
                            nc.gpsimd.dma_gather(
                                Hc[:, :n, :], hsrc,
                                idx_sb[:, c * CH * 8 : (c * CH + n) * 8],
                                num_idxs=n * 128, num_idxs_reg=n * 128, elem_size=D,
                            )
                        # scale all 4 k-blocks for the whole chunk (batched,
                        # 1-port DVE TTs for k0/k1, ACT copy-scale for k2/k3)
                        sC = spool.tile([128, K, CH, D], BF, tag="sc",
                                        name=f"sc_{l}_{c}")
                        for k in range(2):
                            nc.vector.tensor_tensor(
                                sC[:, k, :n, :], Hc[:, :n, :],
                                w_all[:, k, c * CH : c * CH + n].unsqueeze(2)
                                    .broadcast_to([128, n, D]),
                                OP.mult)
                        chunks[c] = (Hc, sC)
                    return chunks[c]

                tbase = 0
                for b in range(min(NT, N_BINS)):
                    Tn = int(T_bins[b])
                    mB = mpool.tile([128, Tn, 128], BF, tag="mask")
                    nc.vector.tensor_tensor(
                        mB[:],
                        iota[:].unsqueeze(1).broadcast_to([128, Tn, 128]),
                        dstloc[:, tbase : tbase + Tn].unsqueeze(2)
                            .broadcast_to([128, Tn, 128]),
                        OP.is_equal,
                    )
                    gp = gpsum.tile([128, K * D], F32, tag="g")
                    for j in range(Tn):
                        t = tbase + j
                        Hc, sC = get_chunk(t)
                        for k in range(2, K):
                            nc.scalar.activation(sC[:, k, t % CH, :],
                                                 Hc[:, t % CH, :], AF.Copy,
                                                 scale=w_all[:, k, t : t + 1])
                        nc.tensor.matmul(gp[:], mB[:, j, :],
                                         sC[:, :, t % CH, :],
                                         start=(j == 0), stop=(j == Tn - 1))
                    # transform: agg = sum_k g_k @ W_k  (+ bias)
                    gsb = opool.tile([128, K, D], BF, tag="gsb")
                    nc.scalar.activation(gsb[:].rearrange("p k d -> p (k d)"),
                                         gp[:], AF.Copy)
                    aggp = apsum.tile([128, D], F32, tag="agg")
                    for k in range(K):
                        gt_ps = tpsum.tile([128, 128], BF, tag="gt")
                        nc.tensor.transpose(gt_ps[:], gsb[:, k, :], ident[:])
                        gt_sb = opool.tile([128, 128], BF, tag="gtsb")
                        nc.vector.tensor_copy(gt_sb[:], gt_ps[:])
                        nc.tensor.matmul(aggp[:], gt_sb[:], fcw[l][:, k, :],
                                         start=(k == 0), stop=(k == 3))
                    rows = min(128, SHARD - b * 128)
                    if l < N_LAYERS - 1:
                        ht = opool.tile([128, D], BF, tag="hout")
                        nc.vector.tensor_tensor(ht[:], aggp[:], biast[l][:], OP.add)
                        nc.sync.dma_start(
                            shard_t[l][b * 128 : b * 128 + rows, :], ht[:rows, :])
                    else:
                        hf = opool.tile([128, D], F32, tag="hfin")
                        nc.vector.tensor_tensor(hf[:], aggp[:], biast[l][:], OP.add)
                        nc.sync.dma_start(
                            out_d[b * 128 : b * 128 + rows, :], hf[:rows, :])
                    tbase += Tn

                if l < 2 and USE_CC:
                    nc.gpsimd.collective_compute(
                        "AllGather", OP.bypass,
                        replica_groups=[list(range(N_CORES))],
                        ins=[shard_t[l].opt()], outs=[hag_t[l].opt()],
                    )
    nc.compile()
    return nc


def _host_inputs(inputs, T_bins, T_tot, plans):
    """Build the 8 per-core input maps."""
    feats = np.zeros((N_NODES, D), dtype=np.float32)
    feats[:, :IN_FEATS] = inputs["features"]
    feat_bf = feats.astype(bf16)
    iota = np.tile(np.arange(128, dtype=np.float32), (128, 1)).astype(bf16)
    ident = np.eye(128, dtype=np.float32).astype(bf16)

    common = {"iota": iota, "ident": ident}
    for l in range(3):
        fc = np.asarray(inputs[f"fc_w{l}"], dtype=np.float32)   # [din, K*128]
        fcp = np.zeros((D, K * D), dtype=np.float32)
        fcp[: fc.shape[0], :] = fc
        fcw = fcp.reshape(D, K, D).astype(bf16)                  # [j, k, o]
        common[f"fcw{l}"] = fcw
        pw = np.asarray(inputs[f"pw{l}"], dtype=np.float32)      # [2,2]
        common[f"pw{l}"] = _rep([pw[0, 0], pw[0, 1], pw[1, 0], pw[1, 1]])
        common[f"pb{l}"] = _rep(inputs[f"pb{l}"])
        common[f"mu{l}"] = _rep(np.asarray(inputs[f"mu{l}"]).reshape(-1))
        common[f"isg{l}"] = _rep(np.asarray(inputs[f"inv_sigma{l}"]).reshape(-1))
        common[f"bias{l}"] = _rep(inputs[f"bias{l}"])

    pseudo = np.asarray(inputs["pseudo"], dtype=np.float32)
    in_maps = []
    for c in range(N_CORES):
        srcP, dstlocP, origP = plans[c]
        m = dict(common)
        m["idx"] = _wrap_idx(srcP)
        # layer-0 source rows pre-gathered into edge order (input sharding)
        m["featP"] = (feat_bf[srcP].reshape(T_tot, 128, D)
                      .transpose(1, 0, 2).copy())
        m["dstloc"] = dstlocP.astype(bf16).reshape(T_tot, 128).T.copy()
        ps = np.zeros((T_tot * 128, 2), dtype=np.float32)
        valid = origP >= 0
        ps[valid] = pseudo[origP[valid]]
        m["pseudo"] = ps.reshape(T_tot, 128, 2).transpose(1, 0, 2).copy()
        in_maps.append(m)
    return in_maps


_CACHE = {}


def _get_compiled(src, dst):
    key = (src.tobytes(), dst.tobytes())
    h = hash(key)
    if h not in _CACHE:
        T_bins, T_tot, plans = _plan_edges(np.asarray(src, dtype=np.int64),
                                           np.asarray(dst, dtype=np.int64))
        nc = build_program(T_bins, T_tot)
        _CACHE[h] = (nc, T_bins, T_tot, plans)
    return _CACHE[h]


def run(inputs, trace=False, **kwargs):
    nc, T_bins, T_tot, plans = _get_compiled(
        np.asarray(inputs["src"]), np.asarray(inputs["dst"]))
    in_maps = _host_inputs(inputs, T_bins, T_tot, plans)
    res = run_bass_kernel_spmd(nc, in_maps, core_ids=list(range(N_CORES)),
                               trace=trace, **kwargs)
    out = np.concatenate([res.results[c]["out"] for c in range(N_CORES)], axis=0)
    return out.astype(np.float32), res


def kernel(**inputs):
    out, _ = run(inputs)
    return out

